# revision 1
# baseline (speedup 1.0000x reference)
"""Trainium2 Bass kernel for nn_CrossAttention_45286135169187.

Math (per batch b, with storage [DIM, HW], tq = w_q*target + b_q [HW]):
    u[c]      = sum_x storage[c,x] * tq[x]
    s         = sum_x tq[x]
    scores[k] = sum_c w_ca[DIM+k, c] * u[c] + b_ca[DIM+k] * s
    attn      = softmax(scores)
    vT[c]     = sum_k attn[k] * w_ca[k, c]
    beta      = sum_k attn[k] * b_ca[k]
    out[d, x] = sum_c vT[c] * storage[c,x] + beta     (identical for all d)

So storage is read exactly once from HBM per pass (held in SBUF across both
passes) and the [B, 2*DIM, HW] conv output never materializes. Sharding:
data-parallel over batch, 2 batches per core across 8 cores; the small
weights are replicated (w_ca is pre-split/transposed on host).
"""

import time

import numpy as np

import concourse.mybir as mybir
import concourse.tile as tile
from concourse import bacc, bass2jax
from concourse.bass import ts

N_CORES = 8
B = 16
DIM = 512
H = 64
W = 64
HW = H * W          # 4096
NB = B // N_CORES   # batches per core = 2
P = 128             # partitions
NCH = DIM // P      # c-chunks = 4
BLK = 512           # x-block (psum bank) size
NBLK = HW // BLK    # 8
F32 = mybir.dt.float32
AX_X = mybir.AxisListType.X
OP_MUL = mybir.AluOpType.mult
OP_ADD = mybir.AluOpType.add
ACT_EXP = mybir.ActivationFunctionType.Exp


def _emit(ctx, tc, ins, out, dbg=None, n_iters=1):
    nc = tc.nc
    storage, target, wkT, wv, bk, bv, wq, bq_col, bq4096 = ins
    BF16 = mybir.dt.bfloat16

    def dump(name, ap):
        if dbg is not None and name in dbg:
            nc.sync.dma_start(out=dbg[name], in_=ap)

    singles = ctx.enter_context(tc.tile_pool(name="singles", bufs=1))
    stf_pool = ctx.enter_context(tc.tile_pool(name="stf", bufs=2))
    stb_pool = ctx.enter_context(tc.tile_pool(name="stb", bufs=6))
    tqb_pool = ctx.enter_context(tc.tile_pool(name="tqb", bufs=1))
    outt_pool = ctx.enter_context(tc.tile_pool(name="outt", bufs=2))
    trow_pool = ctx.enter_context(tc.tile_pool(name="trow", bufs=1))
    small_pool = ctx.enter_context(tc.tile_pool(name="small", bufs=2))
    dram_pool = ctx.enter_context(tc.tile_pool(name="dram", bufs=2, space="DRAM"))
    ps_out = ctx.enter_context(tc.tile_pool(name="ps_out", bufs=4, space="PSUM"))
    ps_small = ctx.enter_context(tc.tile_pool(name="ps_small", bufs=1, space="PSUM"))

    # ---- replicated constants (ACT DMA ring; small ones first so the tq
    # path is unblocked early) ----
    wq_sb = singles.tile([1, 1], F32)
    nc.scalar.dma_start(out=wq_sb, in_=wq)
    bqc_sb = singles.tile([P, 1], F32)
    nc.scalar.dma_start(out=bqc_sb, in_=bq_col)
    bq4096_sb = singles.tile([1, 1], F32)
    nc.scalar.dma_start(out=bq4096_sb, in_=bq4096)
    bk_sb = singles.tile([1, DIM], F32)
    nc.scalar.dma_start(out=bk_sb, in_=bk)
    bv_sb = singles.tile([1, DIM], F32)
    nc.scalar.dma_start(out=bv_sb, in_=bv)
    wv_sb = singles.tile([P, NCH, DIM], F32)    # [p, k-chunk, c]
    wkT_sb = singles.tile([P, NCH, DIM], F32)   # [p, c-chunk, k]

    one_11 = singles.tile([1, 1], F32)          # rhs for row->column transposes
    nc.vector.memset(one_11, 1.0)
    ones_pp = singles.tile([P, P], F32)         # for vT free-dim broadcast
    nc.vector.memset(ones_pp, 1.0)
    scratch = singles.tile([P, HW], F32)        # STT mandatory full-size out sink

    for it in range(n_iters):
        dbg_it = dbg if it == 0 else None

        # ---- per-batch input loads (SP ring) + tq prep + DRAM broadcast ----
        trows, sts, tqbs, s_ts = [], [], [], []
        for b in range(NB):
            trow = trow_pool.tile([1, HW], F32, tag="trow")
            nc.sync.dma_start(out=trow, in_=target[b : b + 1, :])
            trows.append(trow)
        for b in range(NB):
            st = []
            for j in range(NCH):
                t = stf_pool.tile([P, HW], F32, tag="st")
                nc.sync.dma_start(out=t, in_=storage[b, ts(j, P), :])
                st.append(t)
            sts.append(st)

        for b in range(NB):
            # trow <- w_q*target (in place); accum gives sum(w_q*target).
            # +b_q rides into pass 1 as the STT scalar and into s as HW*b_q.
            trow = trows[b]
            sum_twq = small_pool.tile([1, 1], F32, tag="sumtwq")
            nc.vector.tensor_scalar(
                out=trow, in0=trow, scalar1=wq_sb, scalar2=None,
                op0=OP_MUL, op1=OP_ADD, accum_out=sum_twq,
            )
            s_t = small_pool.tile([1, 1], F32, tag="s")
            nc.vector.tensor_scalar(
                out=s_t, in0=sum_twq, scalar1=bq4096_sb, scalar2=None, op0=OP_ADD,
            )
            s_ts.append(s_t)
            # partition-broadcast w_q*target via a DRAM round trip (ACT ring)
            tq_dram = dram_pool.tile([1, HW], F32, tag="tqd")
            nc.scalar.dma_start(out=tq_dram, in_=trow)
            tqb = tqb_pool.tile([P, HW], F32, tag="tqb")
            nc.scalar.dma_start(out=tqb, in_=tq_dram.to_broadcast((P, HW)))
            tqbs.append(tqb)
            if b == 0 and dbg_it:
                dump("dbg_trow", trow)
                dump("dbg_s", s_t)
                dump("dbg_tqb", tqb)

        if it == 0:
            # big weight loads go after the tq stores/reads on the ACT ring so
            # they don't delay the first pass-1 chunk
            nc.scalar.dma_start(
                out=wv_sb, in_=wv.rearrange("(i p) c -> p i c", p=P)
            )
            nc.scalar.dma_start(
                out=wkT_sb, in_=wkT.rearrange("(j p) k -> p j k", p=P)
            )

        for b in range(NB):
            st, tqb, s_t = sts[b], tqbs[b], s_ts[b]

            # bf16 copies of storage for pass 2 (ACT engine; frees f32 slots)
            stb = []
            for j in range(NCH):
                tb = stb_pool.tile([P, HW], BF16, tag="stb")
                nc.scalar.copy(out=tb, in_=st[j])
                stb.append(tb)

            # ---- pass 1: u[c] = <storage[c,:], w_q*target + b_q> ----
            u_t = small_pool.tile([P, NCH], F32, tag="u")
            for j in range(NCH):
                nc.vector.scalar_tensor_tensor(
                    out=scratch, in0=tqb, scalar=bqc_sb, in1=st[j],
                    op0=OP_ADD, op1=OP_MUL, accum_out=u_t[:, j : j + 1],
                )
            if b == 0 and dbg_it:
                dump("dbg_u", u_t)

            # ---- scores row [1, DIM] = u @ wkT + s*bk (PE, accumulated) ----
            pssc = ps_small.tile([1, DIM], F32, tag="scores")
            for j in range(NCH):
                nc.tensor.matmul(
                    pssc, lhsT=u_t[:, j : j + 1], rhs=wkT_sb[:, j, :],
                    start=(j == 0), stop=False,
                )
            nc.tensor.matmul(pssc, lhsT=s_t, rhs=bk_sb, start=False, stop=True)

            # ---- softmax on one partition ----
            negmax = small_pool.tile([1, 1], F32, tag="negmax")
            nc.vector.reduce_max(out=negmax, in_=pssc, axis=AX_X, negate=True)
            attn = small_pool.tile([1, DIM], F32, tag="attn")
            sumexp = small_pool.tile([1, 1], F32, tag="sumexp")
            nc.scalar.activation(
                out=attn, in_=pssc, func=ACT_EXP, bias=negmax, scale=1.0,
                accum_out=sumexp,
            )
            rsum = small_pool.tile([1, 1], F32, tag="rsum")
            nc.vector.reciprocal(out=rsum, in_=sumexp)
            nc.scalar.activation(
                out=attn, in_=attn, func=mybir.ActivationFunctionType.Copy,
                scale=rsum,
            )
            if b == 0 and dbg_it:
                dump("dbg_attn", attn)

            # beta = <attn, bv>; replicate across partitions via the SWDGE
            # ring (overlaps with the attnT/vT matmuls)
            beta = small_pool.tile([1, 1], F32, tag="beta")
            nc.vector.scalar_tensor_tensor(
                out=scratch[0:1, 0:DIM], in0=attn, scalar=1.0, in1=bv_sb,
                op0=OP_MUL, op1=OP_MUL, accum_out=beta,
            )
            beta_dram = dram_pool.tile([1, 1], F32, tag="betad")
            nc.scalar.dma_start(out=beta_dram, in_=beta)
            beta_col = small_pool.tile([P, 1], F32, tag="betac")
            nc.scalar.dma_start(out=beta_col, in_=beta_dram.to_broadcast((P, 1)))

            # ---- attn row -> columns [P, NCH] (PE: lhsT=attn block) ----
            psat = ps_small.tile([P, NCH], F32, tag="attnT")
            for j in range(NCH):
                nc.tensor.matmul(
                    psat[:, j : j + 1], lhsT=attn[:, ts(j, P)], rhs=one_11,
                    start=True, stop=True,
                )
            attnT = small_pool.tile([P, NCH], F32, tag="attnTs")
            nc.vector.tensor_copy(out=attnT, in_=psat)
            if b == 0 and dbg_it:
                dump("dbg_attnT", attnT)

            # ---- vT[c] = sum_k wv[k,c] * attn[k] (PE, natural wv layout) ----
            psvt = ps_small.tile([P, NCH], F32, tag="vT")
            for j in range(NCH):
                for i in range(NCH):
                    nc.tensor.matmul(
                        psvt[:, j : j + 1],
                        lhsT=wv_sb[:, i, ts(j, P)], rhs=attnT[:, i : i + 1],
                        start=(i == 0), stop=(i == NCH - 1),
                    )
            vT = small_pool.tile([P, NCH], F32, tag="vTs")
            nc.vector.tensor_copy(out=vT, in_=psvt)
            if b == 0 and dbg_it:
                dump("dbg_vT", vT)
                dump("dbg_betar", beta_col)

            # broadcast each vT column across 128 stationary columns (bf16)
            vbc = small_pool.tile([P, NCH, P], BF16, tag="vbc")
            for j in range(NCH):
                nc.vector.tensor_scalar_mul(
                    out=vbc[:, j, :], in0=ones_pp, scalar1=vT[:, j : j + 1]
                )

            # ---- pass 2 (bf16): psum[d, x] = sum_c vT[c]*storage[c,x];
            # the DVE copy-out fuses the +beta; stores go on the SWDGE ring ----
            ot = outt_pool.tile([P, HW], F32, tag="ot")
            for blk in range(NBLK):
                pso = ps_out.tile([P, BLK], F32, tag="pso")
                for j in range(NCH):
                    nc.tensor.matmul(
                        pso, lhsT=vbc[:, j, :], rhs=stb[j][:, ts(blk, BLK)],
                        start=(j == 0), stop=(j == NCH - 1),
                    )
                nc.vector.tensor_scalar(
                    out=ot[:, ts(blk, BLK)], in0=pso, scalar1=beta_col,
                    scalar2=None, op0=OP_ADD,
                )

            # the 512 output channels are identical -> write the same tile 4x
            # (in halves, so stores start as soon as half the blocks are out)
            HH = HW // 2
            for half in range(2):
                for dj in range(NCH):
                    nc.gpsimd.dma_start(
                        out=out[b, ts(dj, P), half * HH : (half + 1) * HH],
                        in_=ot[:, half * HH : (half + 1) * HH],
                    )


DBG_SPECS = [
    ("dbg_trow", [1, HW]), ("dbg_s", [1, 1]), ("dbg_tqb", [P, HW]),
    ("dbg_u", [P, NCH]), ("dbg_attn", [1, DIM]), ("dbg_attnT", [P, NCH]),
    ("dbg_vT", [P, NCH]), ("dbg_betar", [1, P]),
]


def _build_program(debug=False, n_iters=1):
    nc = bacc.Bacc(
        "TRN2", target_bir_lowering=False, debug=False, num_devices=N_CORES
    )
    storage = nc.dram_tensor("storage", [NB, DIM, HW], F32, kind="ExternalInput")
    target = nc.dram_tensor("target", [NB, HW], F32, kind="ExternalInput")
    wkT = nc.dram_tensor("wkT", [DIM, DIM], F32, kind="ExternalInput")
    wv = nc.dram_tensor("wv", [DIM, DIM], F32, kind="ExternalInput")
    bk = nc.dram_tensor("bk", [1, DIM], F32, kind="ExternalInput")
    bv = nc.dram_tensor("bv", [1, DIM], F32, kind="ExternalInput")
    wq = nc.dram_tensor("wq", [1, 1], F32, kind="ExternalInput")
    bq_col = nc.dram_tensor("bq_col", [P, 1], F32, kind="ExternalInput")
    bq4096 = nc.dram_tensor("bq4096", [1, 1], F32, kind="ExternalInput")
    out = nc.dram_tensor("out", [NB, DIM, HW], F32, kind="ExternalOutput")
    dbg = None
    if debug:
        dbg = {
            n: nc.dram_tensor(n, s, F32, kind="ExternalOutput").ap()
            for n, s in DBG_SPECS
        }

    from contextlib import ExitStack

    with tile.TileContext(nc) as tc, ExitStack() as ctx:
        _emit(
            ctx,
            tc,
            (
                storage.ap(), target.ap(), wkT.ap(), wv.ap(),
                bk.ap(), bv.ap(), wq.ap(), bq_col.ap(), bq4096.ap(),
            ),
            out.ap(),
            dbg=dbg,
            n_iters=n_iters,
        )
    nc.compile()
    return nc


class _Runner:
    """Jit-once PJRT executor for the compiled Bacc program (8-core SPMD)."""

    def __init__(self, nc):
        import jax
        from jax.experimental.shard_map import shard_map
        from jax.sharding import Mesh, PartitionSpec

        bass2jax.install_neuronx_cc_hook()
        self.jax = jax
        self.nc = nc
        partition_name = (
            nc.partition_id_tensor.name if nc.partition_id_tensor else None
        )
        in_names, out_names, out_avals, zero_outs = [], [], [], []
        for alloc in nc.m.functions[0].allocations:
            if not isinstance(alloc, mybir.MemoryLocationSet):
                continue
            name = alloc.memorylocations[0].name
            if alloc.kind == "ExternalInput":
                if name != partition_name:
                    in_names.append(name)
            elif alloc.kind == "ExternalOutput":
                shape = tuple(alloc.tensor_shape)
                dtype = mybir.dt.np(alloc.dtype)
                out_names.append(name)
                out_avals.append(jax.core.ShapedArray(shape, dtype))
                zero_outs.append(np.zeros(shape, dtype))
        self.in_names, self.out_names = in_names, out_names
        self.n_params = len(in_names)
        n_outs = len(out_avals)

        def _exec(params, out_bufs):
            ops = list(params) + list(out_bufs)
            if partition_name is not None:
                ops.append(bass2jax.partition_id_tensor())
            all_names = tuple(in_names) + tuple(out_names) + (
                (partition_name,) if partition_name else ()
            )
            return bass2jax._bass_exec_p.bind(
                *ops,
                out_avals=tuple(out_avals),
                in_names=all_names,
                out_names=tuple(out_names),
                lowering_input_output_aliases=(),
                sim_require_finite=True,
                sim_require_nnan=True,
                nc=nc,
            )

        def _body(*args):
            return tuple(_exec(args[: self.n_params], args[self.n_params :]))

        devices = jax.devices()[:N_CORES]
        self.mesh = Mesh(np.asarray(devices), ("core",))
        in_specs = (PartitionSpec("core"),) * (self.n_params + n_outs)
        out_specs = (PartitionSpec("core"),) * n_outs
        self.fn = jax.jit(
            shard_map(
                _body, mesh=self.mesh, in_specs=in_specs,
                out_specs=out_specs, check_rep=False,
            ),
            keep_unused=True,
        )
        self.zero_outs = zero_outs
        self._spec = PartitionSpec("core")

    def put_inputs(self, in_maps):
        import jax

        per_core = [[np.asarray(m[n]) for n in self.in_names] for m in in_maps]
        args = [
            np.concatenate([per_core[c][i] for c in range(N_CORES)], axis=0)
            for i in range(self.n_params)
        ]
        args += [np.concatenate([z] * N_CORES, axis=0) for z in self.zero_outs]
        sharding = jax.sharding.NamedSharding(self.mesh, self._spec)
        return [jax.device_put(a, sharding) for a in args]

    def run(self, dev_args):
        outs = self.fn(*dev_args)
        self.jax.block_until_ready(outs)
        return outs

    def results(self, outs):
        res = []
        for c in range(N_CORES):
            d = {}
            for i, name in enumerate(self.out_names):
                arr = np.asarray(outs[i])
                per = arr.shape[0] // N_CORES
                d[name] = arr[c * per : (c + 1) * per]
            res.append(d)
        return res


_CACHE = {}


def _get_runner(n_iters=1):
    key = n_iters
    if key not in _CACHE:
        _CACHE[key] = _Runner(_build_program(n_iters=n_iters))
    return _CACHE[key]


def _make_in_maps(storage, target, w_ca, b_ca, w_q, b_q):
    storage = np.asarray(storage, dtype=np.float32)
    target = np.asarray(target, dtype=np.float32)
    w_ca = np.asarray(w_ca, dtype=np.float32)
    b_ca = np.asarray(b_ca, dtype=np.float32)
    w_q = np.asarray(w_q, dtype=np.float32)
    b_q = np.asarray(b_q, dtype=np.float32)

    # host-side weight prep (tiny): split conv weight into V/K halves,
    # transpose the K half so the contraction dim lands on partitions
    wv = np.ascontiguousarray(w_ca[:DIM])               # [k, c]
    wkT = np.ascontiguousarray(w_ca[DIM:].T)            # [c, k]
    bv = b_ca[:DIM].reshape(1, DIM)
    bk = b_ca[DIM:].reshape(1, DIM)
    wq = w_q.reshape(1, 1)
    bq_col = np.full((P, 1), b_q[0], dtype=np.float32)
    bq4096 = np.array([[b_q[0] * HW]], dtype=np.float32)

    st_flat = storage.reshape(B, DIM, HW)
    tg_flat = target.reshape(B, HW)
    in_maps = []
    for c in range(N_CORES):
        in_maps.append(
            {
                "storage": st_flat[c * NB : (c + 1) * NB],
                "target": tg_flat[c * NB : (c + 1) * NB],
                "wkT": wkT,
                "wv": wv,
                "bk": bk,
                "bv": bv,
                "wq": wq,
                "bq_col": bq_col,
                "bq4096": bq4096,
            }
        )
    return in_maps


def kernel(storage, target, w_ca, b_ca, w_q, b_q):
    runner = _get_runner()
    in_maps = _make_in_maps(storage, target, w_ca, b_ca, w_q, b_q)
    dev_args = runner.put_inputs(in_maps)
    outs = runner.run(dev_args)
    res = runner.results(outs)
    full = np.concatenate([r["out"] for r in res], axis=0)  # [B, DIM, HW]
    return full.reshape(B, DIM, H, W).astype(np.float32)


def time_kernel(storage, target, w_ca, b_ca, w_q, b_q, n_iters=33, reps=8):
    """Estimate per-execution HW time: the NEFF contains the kernel body
    unrolled n_iters times; slope vs the 1-iteration NEFF cancels the
    per-call dispatch overhead."""
    in_maps = _make_in_maps(storage, target, w_ca, b_ca, w_q, b_q)

    def best(runner):
        dev_args = runner.put_inputs(in_maps)
        runner.run(dev_args)  # warm the executable
        times = []
        for _ in range(reps):
            t0 = time.perf_counter()
            runner.run(dev_args)
            times.append(time.perf_counter() - t0)
        return min(times)

    t1 = best(_get_runner(1))
    tn = best(_get_runner(n_iters))
    per_exec = (tn - t1) / (n_iters - 1)
    return per_exec, t1, tn

